# revision 1
# baseline (speedup 1.0000x reference)
"""Trainium2 Bass kernel for nn_AdvancedFastMQA.

Strategy: data-parallel over batch B=8 across the 8 NeuronCores (1 batch
element per core). Everything on-device runs in a *transposed* dataflow so
that no on-device transposes are ever needed:

  xT [in=4096, tok=1024]          (host-transposed, bf16)
  QT[h] = WqT.T @ xT  -> [128(d), 1024(tok)] per head   (PSUM accum over in)
  KT    = WkT.T @ xT  -> [128(d), 1024(tok)]
  V     = xT.T @ WvT  -> [1024(tok), 128(d)]  stored as 8 [128,128] tiles
  RoPE applied to QT/KT in transposed layout (partition-offset DVE ops)
  scoresT[k,q] = KrT_chunk.T @ QrT_window      (contraction over d=128)
  attnT = sigmoid(SCALE * scoresT)             (ACT, PSUM->SBUF, bf16)
  denom[1,q]  = ones.T @ attnT                 (PE accum)
  outT[d,q]   = V_chunk.T @ attnT              (PE accum)
  ao = outT * (1/denom broadcast)              (DVE; gpsimd partition bcast)
  sliding-window overlap blend folded into ao writes
  y[tok, o] = ao.T @ WoT                       (PSUM accum over hd)

Weights are host-side transposed/tiled/bf16-cast so every DMA is contiguous.
"""

import sys
import os

for _p in ("/opt/trn_rl_repo", "/opt/pypackages"):
    if _p not in sys.path:
        sys.path.append(_p)

import numpy as np
import ml_dtypes

import concourse.bacc as bacc
import concourse.tile as tile
import concourse.bass as bass
import concourse.mybir as mybir
from concourse.bass_utils import run_bass_kernel_spmd

BF16 = mybir.dt.bfloat16
F32 = mybir.dt.float32
AF = mybir.ActivationFunctionType

B, S, HD = 8, 1024, 4096
H, DH = 32, 128
WINDOW = 512
SCALE = 1.0 / float(np.sqrt(DH))
ROPE_BASE = 10000.0
NI = HD // 128          # 32 contraction chunks
NT = S // 128           # 8 token chunks
W_STARTS = [0, 256, 512, 768]
W_ENDS = [512, 768, 1024, 1024]

_CACHE = {}


def _rope_cache_np(S_, D_, base=ROPE_BASE):
    inv_freq = 1.0 / (base ** (np.arange(0, D_, 2, dtype=np.float32) / D_))
    t = np.arange(S_, dtype=np.float32)
    f = np.outer(t, inv_freq)
    cos = np.zeros((S_, D_), dtype=np.float32)
    sin = np.zeros((S_, D_), dtype=np.float32)
    cos[:, 0::2] = np.cos(f)
    cos[:, 1::2] = np.cos(f)
    sin[:, 0::2] = np.sin(f)
    sin[:, 1::2] = np.sin(f)
    return cos, sin


def build_nc():
    nc = bacc.Bacc("TRN2", debug=False, target_bir_lowering=False)

    xT_d = nc.dram_tensor("xT", [HD, S], BF16, kind="ExternalInput").ap()
    wq_d = nc.dram_tensor("wq", [H, 128, HD], BF16, kind="ExternalInput").ap()
    wk_d = nc.dram_tensor("wk", [128, HD], BF16, kind="ExternalInput").ap()
    wv_d = nc.dram_tensor("wv", [128, HD], BF16, kind="ExternalInput").ap()
    wo_d = nc.dram_tensor("wo", [8, 4, 128, 8 * 512], BF16, kind="ExternalInput").ap()
    cos_d = nc.dram_tensor("cosT", [128, S], BF16, kind="ExternalInput").ap()
    sin_d = nc.dram_tensor("sinS", [128, S], BF16, kind="ExternalInput").ap()
    alpha_d = nc.dram_tensor("alphaB", [128, 256], BF16, kind="ExternalInput").ap()
    rotm_d = nc.dram_tensor("rotm", [128, 128], BF16, kind="ExternalInput").ap()
    iden_d = nc.dram_tensor("ident", [128, 128], BF16, kind="ExternalInput").ap()
    y_d = nc.dram_tensor("y", [S, HD], F32, kind="ExternalOutput").ap()

    with tile.TileContext(nc) as tc:
        with tc.tile_pool(name="consts", bufs=1) as cp:
            # ---- phase 0: resident loads ----
            wk_t = cp.tile([128, HD], BF16)
            nc.sync.dma_start(out=wk_t[:], in_=wk_d[:])
            wv_t = cp.tile([128, HD], BF16)
            nc.sync.dma_start(out=wv_t[:], in_=wv_d[:])
            xt = cp.tile([128, NI * S], BF16)              # xT tiles, 64KB/part
            for i in range(NI):
                nc.sync.dma_start(
                    out=xt[:, i * S:(i + 1) * S], in_=xT_d[i * 128:(i + 1) * 128, :]
                )
            cos_t = cp.tile([128, S], BF16)
            nc.sync.dma_start(out=cos_t[:], in_=cos_d[:])
            sin_t = cp.tile([128, S], BF16)
            nc.sync.dma_start(out=sin_t[:], in_=sin_d[:])
            alpha_t = cp.tile([128, 256], BF16)
            nc.sync.dma_start(out=alpha_t[:], in_=alpha_d[:])
            ones_t = cp.tile([128, 1], BF16)
            nc.vector.memset(ones_t[:], 1.0)
            rotm_t = cp.tile([128, 128], BF16)
            nc.sync.dma_start(out=rotm_t[:], in_=rotm_d[:])
            iden_t = cp.tile([128, 128], BF16)
            nc.sync.dma_start(out=iden_t[:], in_=iden_d[:])

            kr_t = cp.tile([128, S], BF16)                 # roped K
            v_all = cp.tile([128, NT * 128], BF16)         # V as 8 lhsT tiles
            ao = cp.tile([128, H * S], BF16)               # attention out, 64KB/part

            with tc.tile_pool(name="work", bufs=1) as wp:
              with tc.tile_pool(name="ps", bufs=1, space="PSUM") as pp:

                def rope(dst, src, tag):
                    mc = wp.tile([128, S], BF16, tag="rope_mc", bufs=2)
                    nc.vector.tensor_mul(mc[:], src[:], cos_t[:])
                    for rh in range(2):
                        rp = pp.tile([128, 512], F32, tag="prot", bufs=1)
                        nc.tensor.matmul(
                            rp[:], lhsT=rotm_t[:],
                            rhs=src[:, rh * 512:(rh + 1) * 512],
                            start=True, stop=True,
                        )
                        ms = wp.tile([128, 512], BF16, tag="rope_ms", bufs=2)
                        nc.vector.tensor_mul(ms[:], rp[:], sin_t[:, rh * 512:(rh + 1) * 512])
                        nc.vector.tensor_add(
                            dst[:, rh * 512:(rh + 1) * 512],
                            mc[:, rh * 512:(rh + 1) * 512], ms[:],
                        )

                # ---- phase 1: interleaved K / VT / Q-head0 projections ----
                # one pass over x chunks so PE keeps pace with the x DMA
                wq0_t = wp.tile([128, HD], BF16, tag="wq", bufs=2)
                nc.sync.dma_start(out=wq0_t[:], in_=wq_d[0])
                kps = [pp.tile([128, 512], F32, tag="pscore", bufs=2, name=f"kp{hh}") for hh in range(2)]
                vps = [pp.tile([128, 512], F32, tag="pout", bufs=2, name=f"vp{hh}") for hh in range(2)]
                qps0 = [pp.tile([128, 512], F32, tag="pproj", bufs=2, name=f"qp0{hh}") for hh in range(2)]
                for i in range(NI):
                    st_ = (i == 0)
                    sp_ = (i == NI - 1)
                    for hh in range(2):
                        rhs = xt[:, i * S + hh * 512: i * S + (hh + 1) * 512]
                        nc.tensor.matmul(kps[hh][:], lhsT=wk_t[:, i * 128:(i + 1) * 128],
                                         rhs=rhs, start=st_, stop=sp_)
                        nc.tensor.matmul(vps[hh][:], lhsT=wv_t[:, i * 128:(i + 1) * 128],
                                         rhs=rhs, start=st_, stop=sp_)
                        nc.tensor.matmul(qps0[hh][:], lhsT=wq0_t[:, i * 128:(i + 1) * 128],
                                         rhs=rhs, start=st_, stop=sp_)
                kraw = wp.tile([128, S], BF16, tag="kraw", bufs=1)
                vtraw = wp.tile([128, S], BF16, tag="qrt", bufs=2)
                qraw0 = wp.tile([128, S], BF16, tag="qraw", bufs=2)
                for hh in range(2):
                    nc.scalar.copy(kraw[:, hh * 512:(hh + 1) * 512], kps[hh][:])
                    nc.scalar.copy(vtraw[:, hh * 512:(hh + 1) * 512], vps[hh][:])
                    nc.scalar.copy(qraw0[:, hh * 512:(hh + 1) * 512], qps0[hh][:])
                rope(kr_t, kraw, "k")
                for t in range(NT):
                    tp = pp.tile([128, 128], BF16, tag="prot", bufs=1, name=f"vtp{t}")
                    nc.tensor.transpose(tp[:], vtraw[:, t * 128:(t + 1) * 128], iden_t[:])
                    nc.scalar.copy(v_all[:, t * 128:(t + 1) * 128], tp[:])

                # ---- phase 2: per-head Q proj + rope + attention ----
                for h in range(H):
                    if h == 0:
                        qraw = qraw0
                    else:
                        wq_t = wp.tile([128, HD], BF16, tag="wq", bufs=2)
                        nc.sync.dma_start(out=wq_t[:], in_=wq_d[h])
                        qraw = wp.tile([128, S], BF16, tag="qraw", bufs=2)
                        for half in range(2):
                            ps = pp.tile([128, 512], F32, tag="pproj", bufs=2)
                            for i in range(NI):
                                nc.tensor.matmul(
                                    ps[:],
                                    lhsT=wq_t[:, i * 128:(i + 1) * 128],
                                    rhs=xt[:, i * S + half * 512: i * S + (half + 1) * 512],
                                    start=(i == 0),
                                    stop=(i == NI - 1),
                                )
                            nc.scalar.copy(qraw[:, half * 512:(half + 1) * 512], ps[:])
                    qrt = wp.tile([128, S], BF16, tag="qrt", bufs=2)
                    rope(qrt, qraw, f"q{h}")

                    aoh = ao[:, h * S:(h + 1) * S]
                    for w, (st, en) in enumerate(zip(W_STARTS, W_ENDS)):
                        L = en - st
                        C = L // 128
                        k0 = st // 128
                        atn = []
                        for kc in range(C):
                            sp = pp.tile([128, L], F32, tag="pscore", bufs=2)
                            nc.tensor.matmul(
                                sp[:],
                                lhsT=kr_t[:, (k0 + kc) * 128:(k0 + kc + 1) * 128],
                                rhs=qrt[:, st:en],
                                start=True,
                                stop=True,
                            )
                            at = wp.tile([128, L], BF16, tag="attn", bufs=6)
                            nc.scalar.activation(at[:], sp[:], AF.Sigmoid, scale=SCALE)
                            atn.append(at)
                        dn = pp.tile([1, L], F32, tag="pdenom", bufs=1)
                        for kc in range(C):
                            nc.tensor.matmul(
                                dn[:], lhsT=ones_t[:], rhs=atn[kc][:],
                                start=(kc == 0), stop=(kc == C - 1),
                            )
                        op = pp.tile([128, L], F32, tag="pout", bufs=2)
                        for kc in range(C):
                            nc.tensor.matmul(
                                op[:],
                                lhsT=v_all[:, (k0 + kc) * 128:(k0 + kc + 1) * 128],
                                rhs=atn[kc][:],
                                start=(kc == 0), stop=(kc == C - 1),
                            )
                        rc = wp.tile([1, L], F32, tag="recip", bufs=2)
                        nc.vector.reciprocal_approx_fast(rc[:], dn[:])
                        rb = wp.tile([128, L], F32, tag="recipb", bufs=2)
                        nc.gpsimd.partition_broadcast(rb[:], rc[:])
                        if w == 0:
                            nc.vector.tensor_mul(aoh[:, st:en], op[:], rb[:])
                        else:
                            t1 = wp.tile([128, 256], BF16, tag="bl1", bufs=2)
                            nc.vector.tensor_mul(t1[:], op[:, 0:256], rb[:, 0:256])
                            t2 = wp.tile([128, 256], BF16, tag="bl2", bufs=2)
                            nc.vector.tensor_sub(t2[:], t1[:], aoh[:, st:st + 256])
                            t3 = wp.tile([128, 256], BF16, tag="bl3", bufs=2)
                            nc.vector.tensor_mul(t3[:], t2[:], alpha_t[:])
                            nc.vector.tensor_add(
                                aoh[:, st:st + 256], aoh[:, st:st + 256], t3[:]
                            )
                            if en > st + 256:
                                nc.vector.tensor_mul(
                                    aoh[:, st + 256:en], op[:, 256:L], rb[:, 256:L]
                                )

              # ---- phase 3: output projection (inside work pool: wo
              # slabs share the wq-stream slots so they prefetch during the
              # tail of attention) ----
              if True:
                with tc.tile_pool(name="ops", bufs=1, space="PSUM") as opp:
                    for oc in range(8):
                        yts = []
                        for _t in range(8):
                            ypt = opp.tile([128, 512], F32, tag="yps", bufs=8, name=f"yps{oc}_{_t}")
                            yts.append(ypt)
                        for q4 in range(4):
                            wt = wp.tile([128, 8 * 512], BF16, tag="wq", bufs=2, name=f"wo{oc}_{q4}")
                            nc.sync.dma_start(out=wt[:], in_=wo_d[oc, q4])
                            for tc_ in range(8):
                                for ih in range(8):
                                    i = q4 * 8 + ih
                                    nc.tensor.matmul(
                                        yts[tc_][:],
                                        lhsT=ao[:, i * S + tc_ * 128: i * S + tc_ * 128 + 128],
                                        rhs=wt[:, ih * 512:(ih + 1) * 512],
                                        start=(i == 0),
                                        stop=(i == NI - 1),
                                    )
                        for tc_ in range(8):
                            yt = wp.tile([128, 512], F32, tag=("qraw" if tc_ % 2 == 0 else "qrt"), bufs=2, name=f"ysb{oc}_{tc_}")
                            if tc_ % 2 == 0:
                                nc.scalar.copy(yt[:], yts[tc_][:])
                            else:
                                nc.vector.tensor_copy(yt[:], yts[tc_][:])
                            nc.sync.dma_start(
                                out=y_d[tc_ * 128:(tc_ + 1) * 128, oc * 512:(oc + 1) * 512],
                                in_=yt[:],
                            )
    nc.compile()
    return nc


def prep_inputs(x, Wq, Wk, Wv, Wo):
    """Host-side: transpose/tile/cast so every device DMA is contiguous."""
    bf = ml_dtypes.bfloat16
    xT = np.ascontiguousarray(np.transpose(x, (0, 2, 1))).astype(bf)   # [B,4096,1024]
    # wq[h,p,i*128+c] = Wq[h*128+c, i*128+p]
    wq = np.ascontiguousarray(
        Wq.reshape(H, 128, NI, 128).transpose(0, 3, 2, 1).reshape(H, 128, HD)
    ).astype(bf)
    # wk[p, i*128+c] = Wk[c, i*128+p]
    wk = np.ascontiguousarray(
        Wk.reshape(128, NI, 128).transpose(2, 1, 0).reshape(128, HD)
    ).astype(bf)
    wv = np.ascontiguousarray(
        Wv.reshape(128, NI, 128).transpose(2, 1, 0).reshape(128, HD)
    ).astype(bf)
    # wo[oc,half,p,ih*512+o] = Wo[oc*512+o, (half*16+ih)*128+p]
    wo = np.ascontiguousarray(
        Wo.reshape(8, 512, 4, 8, 128).transpose(0, 2, 4, 3, 1).reshape(8, 4, 128, 8 * 512)
    ).astype(bf)
    cos, sin = _rope_cache_np(S, DH)
    cosT = np.ascontiguousarray(cos.T).astype(bf)                      # [128,1024]
    sinS = np.ascontiguousarray(sin.T).astype(bf)
    rotm = np.zeros((128, 128), dtype=np.float32)
    rotm[np.arange(64) + 64, np.arange(64)] = -1.0
    rotm[np.arange(64), np.arange(64) + 64] = 1.0
    rotm = rotm.astype(bf)
    alphaB = np.tile(
        np.linspace(0.0, 1.0, 256, dtype=np.float32)[None, :], (128, 1)
    ).astype(bf)
    ident = np.eye(128, dtype=np.float32).astype(bf)
    shared = dict(wq=wq, wk=wk, wv=wv, wo=wo, cosT=cosT, sinS=sinS, alphaB=alphaB,
                  rotm=rotm, ident=ident)
    in_maps = [dict(xT=xT[b], **shared) for b in range(B)]
    return in_maps


def kernel(x, Wq, Wk, Wv, Wo):
    if "nc" not in _CACHE:
        _CACHE["nc"] = build_nc()
    nc = _CACHE["nc"]
    in_maps = prep_inputs(
        np.asarray(x, dtype=np.float32),
        np.asarray(Wq, dtype=np.float32),
        np.asarray(Wk, dtype=np.float32),
        np.asarray(Wv, dtype=np.float32),
        np.asarray(Wo, dtype=np.float32),
    )
    res = run_bass_kernel_spmd(nc, in_maps, core_ids=list(range(B)))
    out = np.stack([np.asarray(res.results[b]["y"]) for b in range(B)], axis=0)
    return out.astype(np.float32)


if __name__ == "__main__":
    rng = np.random.default_rng(0)
    x = rng.standard_normal((B, S, HD), dtype=np.float32)
    Wq = (rng.standard_normal((HD, HD), dtype=np.float32) * 0.02)
    Wk = (rng.standard_normal((DH, HD), dtype=np.float32) * 0.02)
    Wv = (rng.standard_normal((DH, HD), dtype=np.float32) * 0.02)
    Wo = (rng.standard_normal((HD, HD), dtype=np.float32) * 0.02)
    y = kernel(x=x, Wq=Wq, Wk=Wk, Wv=Wv, Wo=Wo)
    print("out", y.shape, y.dtype, float(np.abs(y).mean()))



# revision 5
# speedup vs baseline: 1.2684x; 1.2684x over previous
"""Trainium2 Bass kernel for nn_AdvancedFastMQA.

Strategy: data-parallel over batch B=8 across the 8 NeuronCores (1 batch
element per core). Everything on-device runs in a *transposed* dataflow so
that no on-device transposes are ever needed:

  xT [in=4096, tok=1024]          (host-transposed, bf16)
  QT[h] = WqT.T @ xT  -> [128(d), 1024(tok)] per head   (PSUM accum over in)
  KT    = WkT.T @ xT  -> [128(d), 1024(tok)]
  V     = xT.T @ WvT  -> [1024(tok), 128(d)]  stored as 8 [128,128] tiles
  RoPE applied to QT/KT in transposed layout
  scoresT[k,q] = KrT_chunk.T @ QrT_window      (contraction over d=128)
  attnT = sigmoid(SCALE * scoresT)             (ACT, PSUM->SBUF, bf16)

Sliding windows overlap: window w covers q,k in [256w, 256w+512). A (q,k)
score needed by two windows is computed & sigmoided ONCE: attn is stored
per k-chunk over its unique contiguous q-range (window 3 is fully contained
in window 2's tiles). AV matmuls read slices of those tiles.

  denom[1,q] = ones.T @ (sum of the window's 4 attn chunk-tiles)   where the
  chunk pre-reduction runs on DVE, so the PE does ONE N=L ones-matmul per
  window instead of C=4.
  outT[d,q]   = V_chunk.T @ attn_slices        (PE accum)
  ao = outT * (1/denom broadcast)              (DVE; gpsimd partition bcast)
  sliding-window overlap blend folded into ao writes
  y[tok, o] = ao.T @ WoT                       (PSUM accum over hd)

Weights are host-side transposed/tiled/bf16-cast so every DMA is contiguous.
"""

import sys

for _p in ("/opt/trn_rl_repo", "/opt/pypackages"):
    if _p not in sys.path:
        sys.path.append(_p)

import numpy as np
import ml_dtypes

import concourse.bacc as bacc
import concourse.tile as tile
import concourse.bass as bass
import concourse.mybir as mybir
from concourse.bass_utils import run_bass_kernel_spmd

BF16 = mybir.dt.bfloat16
F32 = mybir.dt.float32
AF = mybir.ActivationFunctionType

B, S, HD = 8, 1024, 4096
H, DH = 32, 128
WINDOW = 512
SCALE = 1.0 / float(np.sqrt(DH))
ROPE_BASE = 10000.0
NI = HD // 128          # 32 contraction chunks
NT = S // 128           # 8 token chunks
W_STARTS = [0, 256, 512, 768]
W_ENDS = [512, 768, 1024, 1024]

# attn coverage per k-chunk: kc -> (q_base, q_len); unique q-range each chunk's
# sigmoid output is needed for, across all windows that touch the chunk.
AT_COV = {0: (0, 512), 1: (0, 512), 2: (0, 768), 3: (0, 768),
          4: (256, 768), 5: (256, 768), 6: (512, 512), 7: (512, 512)}
# new score pieces to compute per window: (kc, q_start, q_len)
SC_PIECES = {
    0: [(0, 0, 512), (1, 0, 512), (2, 0, 512), (3, 0, 512)],
    1: [(2, 512, 256), (3, 512, 256), (4, 256, 512), (5, 256, 512)],
    2: [(4, 768, 256), (5, 768, 256), (6, 512, 512), (7, 512, 512)],
    3: [],
}
# AV / denom operands per window: (kc, offset_into_at_tile); slice len == L
AV_OPS = {
    0: [(0, 0), (1, 0), (2, 0), (3, 0)],
    1: [(2, 256), (3, 256), (4, 0), (5, 0)],
    2: [(4, 256), (5, 256), (6, 0), (7, 0)],
    3: [(6, 256), (7, 256)],
}

_CACHE = {}


def _rope_cache_np(S_, D_, base=ROPE_BASE):
    inv_freq = 1.0 / (base ** (np.arange(0, D_, 2, dtype=np.float32) / D_))
    t = np.arange(S_, dtype=np.float32)
    f = np.outer(t, inv_freq)
    cos = np.zeros((S_, D_), dtype=np.float32)
    sin = np.zeros((S_, D_), dtype=np.float32)
    cos[:, 0::2] = np.cos(f)
    cos[:, 1::2] = np.cos(f)
    sin[:, 0::2] = np.sin(f)
    sin[:, 1::2] = np.sin(f)
    return cos, sin


def build_nc():
    nc = bacc.Bacc("TRN2", debug=False, target_bir_lowering=False)

    xT_d = nc.dram_tensor("xT", [HD, S], BF16, kind="ExternalInput").ap()
    wq_d = nc.dram_tensor("wq", [H, 128, HD], BF16, kind="ExternalInput").ap()
    wk_d = nc.dram_tensor("wk", [128, HD], BF16, kind="ExternalInput").ap()
    wv_d = nc.dram_tensor("wv", [128, HD], BF16, kind="ExternalInput").ap()
    wo_d = nc.dram_tensor("wo", [8, 4, 128, 8 * 512], BF16, kind="ExternalInput").ap()
    cos_d = nc.dram_tensor("cosT", [128, S], BF16, kind="ExternalInput").ap()
    sin_d = nc.dram_tensor("sinS", [128, S], BF16, kind="ExternalInput").ap()
    alpha_d = nc.dram_tensor("alphaB", [128, 256], BF16, kind="ExternalInput").ap()
    rotm_d = nc.dram_tensor("rotm", [128, 128], BF16, kind="ExternalInput").ap()
    iden_d = nc.dram_tensor("ident", [128, 128], BF16, kind="ExternalInput").ap()
    y_d = nc.dram_tensor("y", [S, HD], F32, kind="ExternalOutput").ap()

    with tile.TileContext(nc) as tc:
        with tc.tile_pool(name="consts", bufs=1) as cp:
            # ---- persistent resident loads ----
            xt = cp.tile([128, NI * S], BF16)              # xT tiles, 64KB/part
            for i in range(NI):
                nc.sync.dma_start(
                    out=xt[:, i * S:(i + 1) * S], in_=xT_d[i * 128:(i + 1) * 128, :]
                )
            cos_t = cp.tile([128, S], BF16)
            nc.sync.dma_start(out=cos_t[:], in_=cos_d[:])
            sin_t = cp.tile([128, S], BF16)
            nc.sync.dma_start(out=sin_t[:], in_=sin_d[:])
            alpha_t = cp.tile([128, 256], BF16)
            nc.sync.dma_start(out=alpha_t[:], in_=alpha_d[:])
            ones_t = cp.tile([128, 1], BF16)
            nc.vector.memset(ones_t[:], 1.0)
            rotm_t = cp.tile([128, 128], BF16)
            nc.sync.dma_start(out=rotm_t[:], in_=rotm_d[:])
            iden_t = cp.tile([128, 128], BF16)
            nc.sync.dma_start(out=iden_t[:], in_=iden_d[:])

            kr_t = cp.tile([128, S], BF16)                 # roped K
            v_all = cp.tile([128, NT * 128], BF16)         # V as 8 lhsT tiles
            ao = cp.tile([128, H * S], BF16)               # attention out, 64KB/part
            qraw0 = cp.tile([128, S], BF16)                # head-0 Q proj (phase 1)

            def rope(dst, src, wpool, ppool, tag_ps, ps_bufs):
                for rh in range(2):
                    sl = slice(rh * 512, (rh + 1) * 512)
                    mc = wpool.tile([128, 512], BF16, tag="rope_mc", bufs=2, name="mc")
                    nc.vector.tensor_mul(mc[:], src[:, sl], cos_t[:, sl])
                    rp = ppool.tile([128, 512], F32, tag=tag_ps, bufs=ps_bufs, name="rp")
                    nc.tensor.matmul(
                        rp[:], lhsT=rotm_t[:], rhs=src[:, sl], start=True, stop=True
                    )
                    ms = wpool.tile([128, 512], BF16, tag="rope_ms", bufs=2, name="ms")
                    nc.vector.tensor_mul(ms[:], rp[:], sin_t[:, sl])
                    nc.vector.tensor_add(dst[:, sl], mc[:], ms[:])

            # ---- phase 1: interleaved K / VT / Q-head0 projections ----
            with tc.tile_pool(name="p1", bufs=1) as p1:
              with tc.tile_pool(name="ps1", bufs=1, space="PSUM") as pp1:
                wk_t = p1.tile([128, HD], BF16)
                nc.sync.dma_start(out=wk_t[:], in_=wk_d[:])
                wv_t = p1.tile([128, HD], BF16)
                nc.sync.dma_start(out=wv_t[:], in_=wv_d[:])
                wq0_t = p1.tile([128, HD], BF16)
                nc.sync.dma_start(out=wq0_t[:], in_=wq_d[0])

                kps = [pp1.tile([128, 512], F32, tag="p1k", bufs=2, name=f"kp{hh}") for hh in range(2)]
                vps = [pp1.tile([128, 512], F32, tag="p1v", bufs=2, name=f"vp{hh}") for hh in range(2)]
                qps0 = [pp1.tile([128, 512], F32, tag="p1q", bufs=2, name=f"qp0{hh}") for hh in range(2)]
                for i in range(NI):
                    st_ = (i == 0)
                    sp_ = (i == NI - 1)
                    for hh in range(2):
                        rhs = xt[:, i * S + hh * 512: i * S + (hh + 1) * 512]
                        nc.tensor.matmul(kps[hh][:], lhsT=wk_t[:, i * 128:(i + 1) * 128],
                                         rhs=rhs, start=st_, stop=sp_)
                        nc.tensor.matmul(vps[hh][:], lhsT=wv_t[:, i * 128:(i + 1) * 128],
                                         rhs=rhs, start=st_, stop=sp_)
                        nc.tensor.matmul(qps0[hh][:], lhsT=wq0_t[:, i * 128:(i + 1) * 128],
                                         rhs=rhs, start=st_, stop=sp_)
                kraw = p1.tile([128, S], BF16)
                vtraw = p1.tile([128, S], BF16)
                for hh in range(2):
                    nc.scalar.copy(kraw[:, hh * 512:(hh + 1) * 512], kps[hh][:])
                    nc.scalar.copy(vtraw[:, hh * 512:(hh + 1) * 512], vps[hh][:])
                    nc.scalar.copy(qraw0[:, hh * 512:(hh + 1) * 512], qps0[hh][:])
                rope(kr_t, kraw, p1, pp1, "p1rot", 2)
                for t in range(NT):
                    tp = pp1.tile([128, 128], BF16, tag="p1rot", bufs=2, name=f"vtp{t}")
                    nc.tensor.transpose(tp[:], vtraw[:, t * 128:(t + 1) * 128], iden_t[:])
                    nc.scalar.copy(v_all[:, t * 128:(t + 1) * 128], tp[:])

            # ---- phase 2: per-head Q proj + rope + dedup'd attention ----
            with tc.tile_pool(name="work", bufs=1) as wp:
              with tc.tile_pool(name="ps", bufs=1, space="PSUM") as pp:
                wq_tiles = {}
                for h in range(H):
                    # Q projection (head 0 was produced during phase 1)
                    if h == 0:
                        qraw = qraw0
                    else:
                        wq_t = wq_tiles.pop(h)
                        qraw = wp.tile([128, S], BF16, tag="qraw", bufs=2, name="qraw")
                        for half in range(2):
                            ps = pp.tile([128, 512], F32, tag="pproj", bufs=2, name="ps")
                            for i in range(NI):
                                nc.tensor.matmul(
                                    ps[:],
                                    lhsT=wq_t[:, i * 128:(i + 1) * 128],
                                    rhs=xt[:, i * S + half * 512: i * S + (half + 1) * 512],
                                    start=(i == 0),
                                    stop=(i == NI - 1),
                                )
                            nc.scalar.copy(qraw[:, half * 512:(half + 1) * 512], ps[:])
                    # prefetch next head's weights (slot-rotation throttles DMA)
                    if h + 1 < H:
                        wq_n = wp.tile([128, HD], BF16, tag="wq", bufs=2, name="wqt")
                        nc.sync.dma_start(out=wq_n[:], in_=wq_d[h + 1])
                        wq_tiles[h + 1] = wq_n

                    qrt = wp.tile([128, S], BF16, tag="qrt", bufs=2, name="qrt")
                    rope(qrt, qraw, wp, pp, "pscore", 3)

                    # attn tiles for this head, one per k-chunk over its unique
                    # q-range; bufs=16 keeps two heads' sets in flight
                    at = [wp.tile([128, AT_COV[kc][1]], BF16, tag="attn", bufs=16,
                                  name=f"at{kc}") for kc in range(8)]
                    aoh = ao[:, h * S:(h + 1) * S]

                    def scores(w):
                        for (kc, qs, qlen) in SC_PIECES[w]:
                            sp = pp.tile([128, 512], F32, tag="pscore", bufs=3, name="sp")
                            nc.tensor.matmul(
                                sp[:, 0:qlen],
                                lhsT=kr_t[:, kc * 128:(kc + 1) * 128],
                                rhs=qrt[:, qs:qs + qlen],
                                start=True, stop=True,
                            )
                            ab = AT_COV[kc][0]
                            nc.scalar.activation(
                                at[kc][:, qs - ab:qs - ab + qlen], sp[:, 0:qlen],
                                AF.Sigmoid, scale=SCALE,
                            )

                    def av_denom_blend(w):
                        st, en = W_STARTS[w], W_ENDS[w]
                        L = en - st
                        ops = AV_OPS[w]
                        # AV: PSUM accumulate over the window's k chunks
                        op = pp.tile([128, L], F32, tag="pout", bufs=2, name="op")
                        for j, (kc, off) in enumerate(ops):
                            nc.tensor.matmul(
                                op[:],
                                lhsT=v_all[:, kc * 128:(kc + 1) * 128],
                                rhs=at[kc][:, off:off + L],
                                start=(j == 0), stop=(j == len(ops) - 1),
                            )
                        # denominator: DVE chunk pre-reduce, then one ones-matmul
                        partial = wp.tile([128, L], BF16, tag="dnpart", bufs=2, name="dnp")
                        (kc0, off0), (kc1, off1) = ops[0], ops[1]
                        nc.vector.tensor_add(partial[:], at[kc0][:, off0:off0 + L],
                                             at[kc1][:, off1:off1 + L])
                        for (kc, off) in ops[2:]:
                            nc.vector.tensor_add(partial[:], partial[:],
                                                 at[kc][:, off:off + L])
                        dn = pp.tile([1, L], F32, tag="pdenom", bufs=1, name="dn")
                        nc.tensor.matmul(dn[:], lhsT=ones_t[:], rhs=partial[:],
                                         start=True, stop=True)
                        rc = wp.tile([1, L], F32, tag="recip", bufs=2, name="rc")
                        nc.vector.reciprocal_approx_fast(rc[:], dn[:])
                        rb = wp.tile([128, L], F32, tag="recipb", bufs=2, name="rb")
                        nc.gpsimd.partition_broadcast(rb[:], rc[:])
                        if w == 0:
                            nc.vector.tensor_mul(aoh[:, st:en], op[:], rb[:])
                        else:
                            t1 = wp.tile([128, 256], BF16, tag="bl1", bufs=2, name="t1")
                            nc.vector.tensor_mul(t1[:], op[:, 0:256], rb[:, 0:256])
                            t2 = wp.tile([128, 256], BF16, tag="bl2", bufs=2, name="t2")
                            nc.vector.tensor_sub(t2[:], t1[:], aoh[:, st:st + 256])
                            t3 = wp.tile([128, 256], BF16, tag="bl3", bufs=2, name="t3")
                            nc.vector.tensor_mul(t3[:], t2[:], alpha_t[:])
                            nc.vector.tensor_add(
                                aoh[:, st:st + 256], aoh[:, st:st + 256], t3[:]
                            )
                            if en > st + 256:
                                nc.vector.tensor_mul(
                                    aoh[:, st + 256:en], op[:, 256:L], rb[:, 256:L]
                                )

                    # 1-window software pipeline: scores of w+1 are issued
                    # before AV/denom of w so the PE never waits on sigmoid
                    scores(0)
                    scores(1)
                    av_denom_blend(0)
                    scores(2)
                    av_denom_blend(1)
                    av_denom_blend(2)
                    av_denom_blend(3)

              # ---- phase 3: output projection ----
              with tc.tile_pool(name="ops", bufs=1, space="PSUM") as opp:
                    for oc in range(8):
                        yts = []
                        for _t in range(8):
                            ypt = opp.tile([128, 512], F32, tag="yps", bufs=8, name=f"yps{oc}_{_t}")
                            yts.append(ypt)
                        for q4 in range(4):
                            wt = wp.tile([128, 8 * 512], BF16, tag="wq", bufs=2, name=f"wo{oc}_{q4}")
                            nc.sync.dma_start(out=wt[:], in_=wo_d[oc, q4])
                            for tc_ in range(8):
                                for ih in range(8):
                                    i = q4 * 8 + ih
                                    nc.tensor.matmul(
                                        yts[tc_][:],
                                        lhsT=ao[:, i * S + tc_ * 128: i * S + tc_ * 128 + 128],
                                        rhs=wt[:, ih * 512:(ih + 1) * 512],
                                        start=(i == 0),
                                        stop=(i == NI - 1),
                                    )
                        for tc_ in range(8):
                            yt = wp.tile([128, 512], F32, tag=("qraw" if tc_ % 2 == 0 else "qrt"), bufs=2, name=f"ysb{oc}_{tc_}")
                            if tc_ % 2 == 0:
                                nc.scalar.copy(yt[:], yts[tc_][:])
                            else:
                                nc.vector.tensor_copy(yt[:], yts[tc_][:])
                            nc.sync.dma_start(
                                out=y_d[tc_ * 128:(tc_ + 1) * 128, oc * 512:(oc + 1) * 512],
                                in_=yt[:],
                            )
    nc.compile()
    return nc


def prep_inputs(x, Wq, Wk, Wv, Wo):
    """Host-side: transpose/tile/cast so every device DMA is contiguous."""
    bf = ml_dtypes.bfloat16
    xT = np.ascontiguousarray(np.transpose(x, (0, 2, 1))).astype(bf)   # [B,4096,1024]
    # wq[h,p,i*128+c] = Wq[h*128+c, i*128+p]
    wq = np.ascontiguousarray(
        Wq.reshape(H, 128, NI, 128).transpose(0, 3, 2, 1).reshape(H, 128, HD)
    ).astype(bf)
    # wk[p, i*128+c] = Wk[c, i*128+p]
    wk = np.ascontiguousarray(
        Wk.reshape(128, NI, 128).transpose(2, 1, 0).reshape(128, HD)
    ).astype(bf)
    wv = np.ascontiguousarray(
        Wv.reshape(128, NI, 128).transpose(2, 1, 0).reshape(128, HD)
    ).astype(bf)
    # wo[oc,half,p,ih*512+o] = Wo[oc*512+o, (half*16+ih)*128+p]
    wo = np.ascontiguousarray(
        Wo.reshape(8, 512, 4, 8, 128).transpose(0, 2, 4, 3, 1).reshape(8, 4, 128, 8 * 512)
    ).astype(bf)
    cos, sin = _rope_cache_np(S, DH)
    cosT = np.ascontiguousarray(cos.T).astype(bf)                      # [128,1024]
    sinS = np.ascontiguousarray(sin.T).astype(bf)
    rotm = np.zeros((128, 128), dtype=np.float32)
    rotm[np.arange(64) + 64, np.arange(64)] = -1.0
    rotm[np.arange(64), np.arange(64) + 64] = 1.0
    rotm = rotm.astype(bf)
    alphaB = np.tile(
        np.linspace(0.0, 1.0, 256, dtype=np.float32)[None, :], (128, 1)
    ).astype(bf)
    ident = np.eye(128, dtype=np.float32).astype(bf)
    shared = dict(wq=wq, wk=wk, wv=wv, wo=wo, cosT=cosT, sinS=sinS, alphaB=alphaB,
                  rotm=rotm, ident=ident)
    in_maps = [dict(xT=xT[b], **shared) for b in range(B)]
    return in_maps


def kernel(x, Wq, Wk, Wv, Wo):
    if "nc" not in _CACHE:
        _CACHE["nc"] = build_nc()
    nc = _CACHE["nc"]
    in_maps = prep_inputs(
        np.asarray(x, dtype=np.float32),
        np.asarray(Wq, dtype=np.float32),
        np.asarray(Wk, dtype=np.float32),
        np.asarray(Wv, dtype=np.float32),
        np.asarray(Wo, dtype=np.float32),
    )
    res = run_bass_kernel_spmd(nc, in_maps, core_ids=list(range(B)))
    out = np.stack([np.asarray(res.results[b]["y"]) for b in range(B)], axis=0)
    return out.astype(np.float32)


if __name__ == "__main__":
    rng = np.random.default_rng(0)
    x = rng.standard_normal((B, S, HD), dtype=np.float32)
    Wq = (rng.standard_normal((HD, HD), dtype=np.float32) * 0.02)
    Wk = (rng.standard_normal((DH, HD), dtype=np.float32) * 0.02)
    Wv = (rng.standard_normal((DH, HD), dtype=np.float32) * 0.02)
    Wo = (rng.standard_normal((HD, HD), dtype=np.float32) * 0.02)
    y = kernel(x=x, Wq=Wq, Wk=Wk, Wv=Wv, Wo=Wo)
    print("out", y.shape, y.dtype, float(np.abs(y).mean()))
